# revision 1
# baseline (speedup 1.0000x reference)
"""Trainium2 Bass kernel for nn_CLIPCrossProductClassifier.

Math:  y[b,h] = sum_{i,j} img_n[b,i] * txt_n[b,j] * W1r[i,j,h]
       logits = relu(y + b1) @ W2 + b2
with img_n/txt_n the L2-normalized embeddings and W1r = W1.reshape(D,D,H).

Sharding: contraction-parallel over i (rows of the bilinear form). Each of
the 8 cores owns 64 values of i (a [64*D, H] = 32768x512 row-slice of W1,
64 MB) and computes a partial y_c[b,h] = sum_{i in I_c, j} ... . The partials
are summed on the host (8 x 1 MB), followed by the tiny bias/ReLU/[512x1]
projection (0.5 MFLOP of the 137 GFLOP total).

Per-core device schedule (b on PSUM partitions so the img scale is a
per-partition scalar):
  for i in 64:                       # i local to the core
    for b_blk in 4:                  # batch in blocks of 128
      psum[b,h]  = sum_{j_chunk in 4} txtT[j_chunk, b_blk].T @ W1[i, j_chunk, :, :]
      acc[b_blk] = psum * img[b_blk, i] + acc[b_blk]     # ACT scale + DVE add
W1 streams through as the moving operand in float32r (1 cycle/row at N=512,
i.e. full PE rate with ~TF32 multiply precision); txtT is the stationary
operand, reused from SBUF all kernel long. The epilogue is split across the
Scalar engine (scale, PSUM->SBUF) and Vector engine (add) so its ~200 us of
elementwise work hides under the ~232 us matmul stream.
"""

import numpy as np

import concourse.bass as bass
import concourse.tile as tile
from concourse import bacc, mybir
from concourse.bass_utils import run_bass_kernel_spmd

B, D, H = 512, 512, 512
N_CORES = 8
I_PER_CORE = D // N_CORES          # 64
N_BBLK = B // 128                  # 4
N_JCHUNK = D // 128                # 4
EPS = 1e-12

F32 = mybir.dt.float32
F32R = mybir.dt.float32r
F16 = mybir.dt.float16

# Mode -> (txt/stationary dtype, W1/moving dtype, numpy dtypes for each).
# "f32r": ~TF32-precision fast fp32 matmul both sides.
# "f16": half precision both sides (halves DMA, faster LDWEIGHTS).
# "f16w": f16 stationary (fast hidden LDWEIGHTS) + f32r moving (precision).
MM_MODE = "f32r"
_MM_DT = {
    "f32r": (F32R, F32R, np.float32, np.float32),
    "f16": (F16, F16, np.float16, np.float16),
    "f16w": (F16, F32R, np.float16, np.float32),
}

_CACHE = {}


def _l2norm(x: np.ndarray) -> np.ndarray:
    n = np.sqrt(np.sum(x * x, axis=1, keepdims=True, dtype=np.float32))
    return (x / np.maximum(n, np.float32(EPS))).astype(np.float32)


def build_nc(mm=MM_MODE):
    """Build the per-core Bass program (SPMD: same program, per-core data)."""
    txt_dt, w1_dt = _MM_DT[mm][0], _MM_DT[mm][1]
    nc = bacc.Bacc(
        "TRN2",
        target_bir_lowering=False,
        debug=False,
        num_devices=N_CORES,
    )

    txt_t = nc.dram_tensor("txt_t", [D, B], txt_dt, kind="ExternalInput").ap()
    img_s = nc.dram_tensor("img_s", [B, I_PER_CORE], F32, kind="ExternalInput").ap()
    w1_s = nc.dram_tensor(
        "w1_s", [I_PER_CORE, N_JCHUNK, 128, H], w1_dt, kind="ExternalInput"
    ).ap()
    yp = nc.dram_tensor("yp", [B, H], F32, kind="ExternalOutput").ap()

    with tile.TileContext(nc) as tc:
        with (
            tc.tile_pool(name="const", bufs=1) as constp,
            tc.tile_pool(name="w1", bufs=6) as w1p,
            tc.tile_pool(name="accs", bufs=1) as accp,
            tc.tile_pool(name="scl", bufs=6) as sclp,
            tc.tile_pool(name="ps", bufs=6, space=bass.MemorySpace.PSUM) as psump,
        ):
            # Prefetch the i=0 weight slab before anything else: the first
            # matmul group needs it together with the txt tiles below.
            w1t0 = [
                w1p.tile([128, H], w1_dt, tag=f"w1c{c}", name=f"w1c{c}p")
                for c in range(N_JCHUNK)
            ]
            for c in range(N_JCHUNK):
                nc.sync.dma_start(w1t0[c][:], w1_s[0, c])

            # txt stationary, split into two b-halves per j-chunk so the first
            # matmuls only wait on half the 1 MB transpose load.
            txt_sb = []  # [c][half] -> [128, 256]
            for c in range(N_JCHUNK):
                halves = []
                for hh in range(2):
                    t = constp.tile(
                        [128, B // 2], txt_dt,
                        tag=f"txt{c}h{hh}", name=f"txt_sb{c}h{hh}",
                    )
                    nc.sync.dma_start(
                        t[:],
                        txt_t[c * 128 : (c + 1) * 128,
                              hh * (B // 2) : (hh + 1) * (B // 2)],
                    )
                    halves.append(t)
                txt_sb.append(halves)
            img_sb = []
            for bb in range(N_BBLK):
                t = constp.tile([128, I_PER_CORE], F32, tag=f"img{bb}", name=f"img_sb{bb}")
                nc.sync.dma_start(t[:], img_s[bb * 128 : (bb + 1) * 128, :])
                img_sb.append(t)
            acc = [
                accp.tile([128, H], F32, tag=f"acc{bb}", name=f"acc{bb}")
                for bb in range(N_BBLK)
            ]

            for i in range(I_PER_CORE):
                # One 256 KB tile per j-chunk: each matmul depends only on its
                # own quarter of the per-i weight slab. i=0 was prefetched.
                if i == 0:
                    w1t = w1t0
                else:
                    w1t = [
                        w1p.tile([128, H], w1_dt, tag=f"w1c{c}", name=f"w1c{c}")
                        for c in range(N_JCHUNK)
                    ]
                    for c in range(N_JCHUNK):
                        nc.sync.dma_start(w1t[c][:], w1_s[i, c])
                for bb in range(N_BBLK):
                    ps = psump.tile([128, H], F32, tag="ps")
                    for c in range(N_JCHUNK):
                        lhs = txt_sb[c][bb // 2]
                        col = (bb % 2) * 128
                        nc.tensor.matmul(
                            ps[:],
                            lhs[:, col : col + 128],
                            w1t[c][:],
                            start=(c == 0),
                            stop=(c == N_JCHUNK - 1),
                        )
                    sc = img_sb[bb][:, i : i + 1]
                    if i == 0:
                        # ACT writes the scaled first term straight into acc.
                        nc.scalar.activation(
                            acc[bb][:], ps[:], mybir.ActivationFunctionType.Copy,
                            scale=sc,
                        )
                    else:
                        # ACT: scaled = psum * img scalar; DVE: acc += scaled.
                        scaled = sclp.tile([128, H], F32, tag="scaled", name="scaled")
                        nc.scalar.activation(
                            scaled[:], ps[:], mybir.ActivationFunctionType.Copy,
                            scale=sc,
                        )
                        nc.vector.tensor_add(acc[bb][:], acc[bb][:], scaled[:])

            for bb in range(N_BBLK):
                nc.sync.dma_start(yp[bb * 128 : (bb + 1) * 128, :], acc[bb][:])

    nc.compile()
    return nc


def make_in_maps(image_embeds, text_embeds, W1, mm=MM_MODE):
    txt_np, w1_np = _MM_DT[mm][2], _MM_DT[mm][3]
    imgn = _l2norm(np.asarray(image_embeds, np.float32))
    txtn = _l2norm(np.asarray(text_embeds, np.float32))
    txt_t = np.ascontiguousarray(txtn.T).astype(txt_np)
    W1r = np.asarray(W1, np.float32).reshape(D, D, H).astype(w1_np)
    in_maps = []
    for c in range(N_CORES):
        w1c = W1r[c * I_PER_CORE : (c + 1) * I_PER_CORE].reshape(
            I_PER_CORE, N_JCHUNK, 128, H
        )
        in_maps.append(
            {
                "txt_t": txt_t,
                "img_s": np.ascontiguousarray(imgn[:, c * I_PER_CORE : (c + 1) * I_PER_CORE]),
                "w1_s": w1c,
            }
        )
    return in_maps


def run_device(in_maps, trace=False, mm=MM_MODE, **kw):
    if mm not in _CACHE:
        _CACHE[mm] = build_nc(mm)
    return run_bass_kernel_spmd(
        _CACHE[mm], in_maps, list(range(N_CORES)), trace=trace, **kw
    )


def finish_host(results, b1, W2, b2):
    Y = np.zeros((B, H), np.float32)
    for c in range(N_CORES):
        Y += results[c]["yp"]
    h = np.maximum(Y + np.asarray(b1, np.float32), np.float32(0.0))
    out = h @ np.asarray(W2, np.float32) + np.asarray(b2, np.float32)
    return out.astype(np.float32)


def kernel(image_embeds, text_embeds, W1, b1, W2, b2):
    in_maps = make_in_maps(image_embeds, text_embeds, W1)
    res = run_device(in_maps, trace=False)
    return finish_host(res.results, b1, W2, b2)



# revision 3
# speedup vs baseline: 1.0391x; 1.0391x over previous
"""Trainium2 Bass kernel for nn_CLIPCrossProductClassifier.

Math:  y[b,h] = sum_{i,j} img_n[b,i] * txt_n[b,j] * W1r[i,j,h]
       logits = relu(y + b1) @ W2 + b2
with img_n/txt_n the L2-normalized embeddings and W1r = W1.reshape(D,D,H).

Sharding: contraction-parallel over i (rows of the bilinear form). Each of
the 8 cores owns 64 values of i (a [64*D, H] = 32768x512 row-slice of W1,
32 MB in f16) and computes a partial y_c[b,h] = sum_{i in I_c, j} ... . The
partials are summed on the host (8 x 1 MB), followed by the tiny
bias/ReLU/[512x1] projection (0.5 MFLOP of the 137 GFLOP total).

Per-core device schedule (b on PSUM partitions so the img scale is a
per-partition scalar):
  for i in 64:                       # i local to the core
    for b_blk in 4:                  # batch in blocks of 128
      psum[b,h]  = sum_{j_chunk in 4} txtT[j_chunk, b_blk].T @ W1[i, j_chunk, :, :]
      acc[b_blk] = psum * img[b_blk, i] + acc[b_blk]     # ACT scale + DVE add
W1 streams through as the moving operand in f16 (1 cycle/row at N=512, full
PE rate, and half the HBM traffic of f32); txtT is the stationary operand
(f16 -> fast weight load), reused from SBUF all kernel long. W1 is
pre-scaled by 512 on the host (sigma ~1 in f16, no subnormals) and the
1/512 is folded into the per-partition img scale. The epilogue is split
across the Scalar engine (scale, PSUM->SBUF) and Vector engine (add) so its
elementwise work hides under the matmul stream.

Startup: a block of dummy N=128 matmuls on a zeroed SBUF tile runs while
the input DMAs land, keeping the PE HAM warm (2.4 GHz) so the real matmul
stream starts at full rate; txt tiles are DMA'd ahead of the W1 stream so
the first real matmul's stationary operand arrives first.
"""

import numpy as np

import concourse.bass as bass
import concourse.tile as tile
from concourse import bacc, mybir
from concourse.bass_utils import run_bass_kernel_spmd

B, D, H = 512, 512, 512
N_CORES = 8
I_PER_CORE = D // N_CORES          # 64
N_BBLK = B // 128                  # 4
N_JCHUNK = D // 128                # 4
EPS = 1e-12
W1_SCALE = 512.0                   # power of two: exact in fp, folded into img

F32 = mybir.dt.float32
F32R = mybir.dt.float32r
F16 = mybir.dt.float16

# Mode -> (txt/stationary dtype, W1/moving dtype, numpy dtypes for each).
MM_MODE = "f16"
_MM_DT = {
    "f32r": (F32R, F32R, np.float32, np.float32),
    "f16": (F16, F16, np.float16, np.float16),
    "f16w": (F16, F32R, np.float16, np.float32),
}

N_WARM = 40                         # dummy matmuls to keep HAM warm at start

_CACHE = {}


def _l2norm(x: np.ndarray) -> np.ndarray:
    n = np.sqrt(np.sum(x * x, axis=1, keepdims=True, dtype=np.float32))
    return (x / np.maximum(n, np.float32(EPS))).astype(np.float32)


def build_nc(mm=MM_MODE):
    """Build the per-core Bass program (SPMD: same program, per-core data)."""
    txt_dt, w1_dt = _MM_DT[mm][0], _MM_DT[mm][1]
    nc = bacc.Bacc(
        "TRN2",
        target_bir_lowering=False,
        debug=False,
        num_devices=N_CORES,
    )

    txt_t = nc.dram_tensor("txt_t", [D, B], txt_dt, kind="ExternalInput").ap()
    img_s = nc.dram_tensor("img_s", [B, I_PER_CORE], F32, kind="ExternalInput").ap()
    w1_s = nc.dram_tensor(
        "w1_s", [I_PER_CORE, N_JCHUNK, 128, H], w1_dt, kind="ExternalInput"
    ).ap()
    yp = nc.dram_tensor("yp", [B, H], F32, kind="ExternalOutput").ap()

    with tile.TileContext(nc) as tc:
        with (
            tc.tile_pool(name="warm", bufs=1) as warmp,
            tc.tile_pool(name="warmps", bufs=2, space=bass.MemorySpace.PSUM) as warmpsp,
            tc.tile_pool(name="const", bufs=1) as constp,
            tc.tile_pool(name="w1", bufs=10) as w1p,
            tc.tile_pool(name="accs", bufs=1) as accp,
            tc.tile_pool(name="scl", bufs=6) as sclp,
            tc.tile_pool(name="ps", bufs=6, space=bass.MemorySpace.PSUM) as psump,
        ):
            # txt stationary first in the DMA queues: the first matmuls need
            # it before any W1. Split into two b-halves per j-chunk so the
            # first matmuls only wait on half the transpose load.
            txt_sb = []  # [c][half] -> [128, 256]
            for c in range(N_JCHUNK):
                halves = []
                for hh in range(2):
                    t = constp.tile(
                        [128, B // 2], txt_dt,
                        tag=f"txt{c}h{hh}", name=f"txt_sb{c}h{hh}",
                    )
                    nc.sync.dma_start(
                        t[:],
                        txt_t[c * 128 : (c + 1) * 128,
                              hh * (B // 2) : (hh + 1) * (B // 2)],
                    )
                    halves.append(t)
                txt_sb.append(halves)

            # First two i-slabs of W1 next.
            w1t0 = [
                w1p.tile([128, H], w1_dt, tag=f"w1c{c}", name=f"w1c{c}p")
                for c in range(N_JCHUNK)
            ]
            for c in range(N_JCHUNK):
                nc.sync.dma_start(w1t0[c][:], w1_s[0, c])

            img_sb = []
            for bb in range(N_BBLK):
                t = constp.tile([128, I_PER_CORE], F32, tag=f"img{bb}", name=f"img_sb{bb}")
                nc.sync.dma_start(t[:], img_s[bb * 128 : (bb + 1) * 128, :])
                img_sb.append(t)
            acc = [
                accp.tile([128, H], F32, tag=f"acc{bb}", name=f"acc{bb}")
                for bb in range(N_BBLK)
            ]

            # Warm-up: dummy matmuls on a zeroed tile keep the PE busy (and
            # the HAM un-throttled) while the real input DMAs land. They sit
            # ahead of the real matmuls on the tensor queue and have no data
            # dependencies beyond the one memset.
            wz = warmp.tile([128, 256], txt_dt, tag="wz", name="warm_zero")
            nc.vector.memset(wz[:], 0.0)
            for k in range(N_WARM):
                wps = warmpsp.tile([128, 128], F32, tag="wps")
                nc.tensor.matmul(
                    wps[:], wz[:, :128], wz[:, 128:], start=True, stop=True
                )

            for i in range(I_PER_CORE):
                # One tile per j-chunk: each matmul depends only on its own
                # quarter of the per-i weight slab. i=0 was prefetched.
                if i == 0:
                    w1t = w1t0
                else:
                    w1t = [
                        w1p.tile([128, H], w1_dt, tag=f"w1c{c}", name=f"w1c{c}")
                        for c in range(N_JCHUNK)
                    ]
                    for c in range(N_JCHUNK):
                        nc.sync.dma_start(w1t[c][:], w1_s[i, c])
                for bb in range(N_BBLK):
                    ps = psump.tile([128, H], F32, tag="ps")
                    for c in range(N_JCHUNK):
                        lhs = txt_sb[c][bb // 2]
                        col = (bb % 2) * 128
                        nc.tensor.matmul(
                            ps[:],
                            lhs[:, col : col + 128],
                            w1t[c][:],
                            start=(c == 0),
                            stop=(c == N_JCHUNK - 1),
                        )
                    sc = img_sb[bb][:, i : i + 1]
                    if i == 0:
                        # ACT writes the scaled first term straight into acc.
                        nc.scalar.activation(
                            acc[bb][:], ps[:], mybir.ActivationFunctionType.Copy,
                            scale=sc,
                        )
                    else:
                        # ACT: scaled = psum * img scalar; DVE: acc += scaled.
                        scaled = sclp.tile([128, H], F32, tag="scaled", name="scaled")
                        nc.scalar.activation(
                            scaled[:], ps[:], mybir.ActivationFunctionType.Copy,
                            scale=sc,
                        )
                        nc.vector.tensor_add(acc[bb][:], acc[bb][:], scaled[:])

            for bb in range(N_BBLK):
                nc.sync.dma_start(yp[bb * 128 : (bb + 1) * 128, :], acc[bb][:])

    nc.compile()
    return nc


def make_in_maps(image_embeds, text_embeds, W1, mm=MM_MODE):
    txt_np, w1_np = _MM_DT[mm][2], _MM_DT[mm][3]
    imgn = _l2norm(np.asarray(image_embeds, np.float32))
    txtn = _l2norm(np.asarray(text_embeds, np.float32))
    txt_t = np.ascontiguousarray(txtn.T).astype(txt_np)
    W1r = np.asarray(W1, np.float32).reshape(D, D, H)
    if w1_np != np.float32:
        # Pre-scale so sigma(W1) ~ 1: keeps everything in f16 normal range.
        W1r = (W1r * np.float32(W1_SCALE)).astype(w1_np)
        imgn = imgn * np.float32(1.0 / W1_SCALE)
    in_maps = []
    for c in range(N_CORES):
        w1c = W1r[c * I_PER_CORE : (c + 1) * I_PER_CORE].reshape(
            I_PER_CORE, N_JCHUNK, 128, H
        )
        in_maps.append(
            {
                "txt_t": txt_t,
                "img_s": np.ascontiguousarray(imgn[:, c * I_PER_CORE : (c + 1) * I_PER_CORE]),
                "w1_s": w1c,
            }
        )
    return in_maps


def run_device(in_maps, trace=False, mm=MM_MODE, **kw):
    if mm not in _CACHE:
        _CACHE[mm] = build_nc(mm)
    return run_bass_kernel_spmd(
        _CACHE[mm], in_maps, list(range(N_CORES)), trace=trace, **kw
    )


def finish_host(results, b1, W2, b2):
    Y = np.zeros((B, H), np.float32)
    for c in range(N_CORES):
        Y += results[c]["yp"]
    h = np.maximum(Y + np.asarray(b1, np.float32), np.float32(0.0))
    out = h @ np.asarray(W2, np.float32) + np.asarray(b2, np.float32)
    return out.astype(np.float32)


def kernel(image_embeds, text_embeds, W1, b1, W2, b2):
    in_maps = make_in_maps(image_embeds, text_embeds, W1)
    res = run_device(in_maps, trace=False)
    return finish_host(res.results, b1, W2, b2)


# revision 4
# speedup vs baseline: 1.0675x; 1.0273x over previous
"""Trainium2 Bass kernel for nn_CLIPCrossProductClassifier.

Math:  y[b,h] = sum_{i,j} img_n[b,i] * txt_n[b,j] * W1r[i,j,h]
       logits = relu(y + b1) @ W2 + b2
with img_n/txt_n the L2-normalized embeddings and W1r = W1.reshape(D,D,H).

Sharding: contraction-parallel over i (rows of the bilinear form). Each of
the 8 cores owns 64 values of i (a [64*D, H] row-slice of W1, 32 MB in f16)
and computes a partial y_c[b,h] = sum_{i in I_c, j} ... . The partials are
summed on the host (8 x 1 MB), followed by the tiny bias/ReLU/[512x1]
projection (0.5 MFLOP of the 137 GFLOP total).

Per-core device schedule (b on PSUM partitions so the img scale is a
per-partition scalar):
  for i in 64:                       # i local to the core
    for b_blk in 4:                  # batch in blocks of 128
      psum[b,h] = sum_{j_chunk in 4} txtT[j_chunk, b_blk].T @ W1[i, j_chunk, :, :]
      acc[b_blk] = psum * img[b_blk, i] + acc[b_blk]   # one fused DVE op
W1 streams through as the moving operand in f16 (1 cycle/row at N=512 =
full PE rate, half the HBM bytes of f32); txtT is the stationary operand
(f16 -> fast weight load). W1 is pre-scaled by 512 on the host (sigma ~1,
no f16 subnormals); the 1/512 is folded into the per-partition img scale.

DMA layout: every SBUF row is >= 2 KB so the DMA moves full-size packets
(the f32 baseline was packet-rate-bound at ~1 KB/packet):
 - W1 per i as one [128, 2048] f16 tile (row r = W1[i, {r,128+r,256+r,384+r}, :])
 - txt as one [128, 2048] f16 tile (row r = txtT[{r,128+r,256+r,384+r}, :])
 - img as one [128, 256] f32 tile (row r = img[{r,128+r,256+r,384+r}, :]/512)

Epilogue is a single fused InstTensorScalarPtr per (i, b_blk) on the Vector
engine: acc = (psum * img_scalar) + acc, reading PSUM directly.

Startup: dummy matmuls on a zeroed tile keep the PE HAM un-throttled
(2.4 GHz) while the first input DMAs land; txt is first in the DMA queues.
"""

import numpy as np

import concourse.bass as bass
import concourse.tile as tile
from concourse import bacc, mybir
from concourse.bass_utils import run_bass_kernel_spmd

B, D, H = 512, 512, 512
N_CORES = 8
I_PER_CORE = D // N_CORES          # 64
N_BBLK = B // 128                  # 4
N_JCHUNK = D // 128                # 4
EPS = 1e-12
W1_SCALE = 512.0                   # power of two: exact in fp, folded into img

F32 = mybir.dt.float32
F16 = mybir.dt.float16

N_WARM = 12                        # dummy matmuls to keep HAM warm at start

_CACHE = {}


def _l2norm(x: np.ndarray) -> np.ndarray:
    n = np.sqrt(np.sum(x * x, axis=1, keepdims=True, dtype=np.float32))
    return (x / np.maximum(n, np.float32(EPS))).astype(np.float32)


def build_nc():
    """Build the per-core Bass program (SPMD: same program, per-core data)."""
    nc = bacc.Bacc(
        "TRN2",
        target_bir_lowering=False,
        debug=False,
        num_devices=N_CORES,
    )

    txt_p = nc.dram_tensor("txt_p", [128, N_JCHUNK * B], F16, kind="ExternalInput").ap()
    img_p = nc.dram_tensor(
        "img_p", [128, N_BBLK * I_PER_CORE], F32, kind="ExternalInput"
    ).ap()
    w1_s = nc.dram_tensor(
        "w1_s", [I_PER_CORE, 128, N_JCHUNK * H], F16, kind="ExternalInput"
    ).ap()
    yp = nc.dram_tensor("yp", [B, H], F32, kind="ExternalOutput").ap()

    with tile.TileContext(nc) as tc:
        with (
            tc.tile_pool(name="warm", bufs=1) as warmp,
            tc.tile_pool(name="warmps", bufs=2, space=bass.MemorySpace.PSUM) as warmpsp,
            tc.tile_pool(name="const", bufs=1) as constp,
            tc.tile_pool(name="w1", bufs=8) as w1p,
            tc.tile_pool(name="accs", bufs=1) as accp,
            tc.tile_pool(name="ps", bufs=6, space=bass.MemorySpace.PSUM) as psump,
        ):
            # txt stationary first in the DMA queues: the first matmuls need
            # it before any W1.
            txt_sb = constp.tile([128, N_JCHUNK * B], F16, tag="txt", name="txt_sb")
            nc.sync.dma_start(txt_sb[:], txt_p[:, :])

            # First i-slab of W1 next (one quad tile, 4 KB rows).
            w1t0 = w1p.tile([128, N_JCHUNK * H], F16, tag="w1", name="w1p0")
            nc.sync.dma_start(w1t0[:], w1_s[0])

            img_sb = constp.tile(
                [128, N_BBLK * I_PER_CORE], F32, tag="img", name="img_sb"
            )
            nc.sync.dma_start(img_sb[:], img_p[:, :])

            acc = [
                accp.tile([128, H], F32, tag=f"acc{bb}", name=f"acc{bb}")
                for bb in range(N_BBLK)
            ]

            # Warm-up: dummy matmuls on a zeroed tile keep the PE busy (and
            # the HAM un-throttled) while the real input DMAs land. They sit
            # ahead of the real matmuls on the tensor queue and have no data
            # dependencies beyond the one memset.
            wz = warmp.tile([128, 384], F16, tag="wz", name="warm_zero")
            nc.vector.memset(wz[:], 0.0)
            for k in range(N_WARM):
                wps = warmpsp.tile([128, 256], F32, tag="wps")
                nc.tensor.matmul(
                    wps[:], wz[:, :128], wz[:, 128:384], start=True, stop=True
                )

            for i in range(I_PER_CORE):
                if i == 0:
                    w1t = w1t0
                else:
                    w1t = w1p.tile([128, N_JCHUNK * H], F16, tag="w1", name="w1")
                    nc.sync.dma_start(w1t[:], w1_s[i])
                for bb in range(N_BBLK):
                    ps = psump.tile([128, H], F32, tag="ps")
                    for c in range(N_JCHUNK):
                        nc.tensor.matmul(
                            ps[:],
                            txt_sb[:, c * B + bb * 128 : c * B + (bb + 1) * 128],
                            w1t[:, c * H : (c + 1) * H],
                            start=(c == 0),
                            stop=(c == N_JCHUNK - 1),
                        )
                    sc = img_sb[:, bb * I_PER_CORE + i : bb * I_PER_CORE + i + 1]
                    if i == 0:
                        # acc = psum * img  (fused, Vector engine)
                        nc.vector.tensor_scalar_mul(acc[bb][:], ps[:], sc)
                    else:
                        # acc = (psum * img) + acc  (one fused DVE op)
                        nc.vector.scalar_tensor_tensor(
                            acc[bb][:], ps[:], sc, acc[bb][:],
                            mybir.AluOpType.mult, mybir.AluOpType.add,
                        )

            for bb in range(N_BBLK):
                nc.sync.dma_start(yp[bb * 128 : (bb + 1) * 128, :], acc[bb][:])

    nc.compile()
    return nc


def make_in_maps(image_embeds, text_embeds, W1):
    imgn = _l2norm(np.asarray(image_embeds, np.float32)) * np.float32(1.0 / W1_SCALE)
    txtn = _l2norm(np.asarray(text_embeds, np.float32))
    # txt packed: row r = [txtT[r], txtT[128+r], txtT[256+r], txtT[384+r]]
    txt_t = np.ascontiguousarray(txtn.T).astype(np.float16)          # [D, B]
    txt_p = np.ascontiguousarray(
        txt_t.reshape(N_JCHUNK, 128, B).transpose(1, 0, 2).reshape(128, N_JCHUNK * B)
    )
    W1r = (np.asarray(W1, np.float32).reshape(D, D, H) * np.float32(W1_SCALE)).astype(
        np.float16
    )
    in_maps = []
    for c in range(N_CORES):
        # img packed per core: row r = [img[r, Ic], img[128+r, Ic], ...]
        ic = np.ascontiguousarray(imgn[:, c * I_PER_CORE : (c + 1) * I_PER_CORE])
        img_pk = np.ascontiguousarray(
            ic.reshape(N_BBLK, 128, I_PER_CORE)
            .transpose(1, 0, 2)
            .reshape(128, N_BBLK * I_PER_CORE)
        )
        # W1 per i: row r = [W1[i, r, :], W1[i, 128+r, :], W1[i, 256+r, :], W1[i, 384+r, :]]
        w1c = W1r[c * I_PER_CORE : (c + 1) * I_PER_CORE]             # [64, D, H]
        w1pk = np.ascontiguousarray(
            w1c.reshape(I_PER_CORE, N_JCHUNK, 128, H)
            .transpose(0, 2, 1, 3)
            .reshape(I_PER_CORE, 128, N_JCHUNK * H)
        )
        in_maps.append({"txt_p": txt_p, "img_p": img_pk, "w1_s": w1pk})
    return in_maps


def run_device(in_maps, trace=False, **kw):
    if "nc" not in _CACHE:
        _CACHE["nc"] = build_nc()
    return run_bass_kernel_spmd(
        _CACHE["nc"], in_maps, list(range(N_CORES)), trace=trace, **kw
    )


def finish_host(results, b1, W2, b2):
    Y = np.zeros((B, H), np.float32)
    for c in range(N_CORES):
        Y += results[c]["yp"]
    h = np.maximum(Y + np.asarray(b1, np.float32), np.float32(0.0))
    out = h @ np.asarray(W2, np.float32) + np.asarray(b2, np.float32)
    return out.astype(np.float32)


def kernel(image_embeds, text_embeds, W1, b1, W2, b2):
    in_maps = make_in_maps(image_embeds, text_embeds, W1)
    res = run_device(in_maps, trace=False)
    return finish_host(res.results, b1, W2, b2)


# revision 9
# speedup vs baseline: 1.0798x; 1.0114x over previous
"""Trainium2 Bass kernel for nn_CLIPCrossProductClassifier.

Math:  y[b,h] = sum_{i,j} img_n[b,i] * txt_n[b,j] * W1r[i,j,h]
       logits = relu(y + b1) @ W2 + b2
with img_n/txt_n the L2-normalized embeddings and W1r = W1.reshape(D,D,H).

Sharding: contraction-parallel over i (rows of the bilinear form). Each of
the 8 cores owns 64 values of i (a [64*D, H] row-slice of W1, 32 MB in f16)
and computes a partial y_c[b,h] = sum_{i in I_c, j} ... . The partials are
summed on the host (8 x 1 MB), followed by the tiny bias/ReLU/[512x1]
projection (0.5 MFLOP of the 137 GFLOP total).

Per-core device schedule (b on PSUM partitions so the img scale is a
per-partition scalar):
  for i in 64:                       # i local to the core
    for b_blk in 4:                  # batch in blocks of 128
      psum[b,h] = sum_{j_chunk in 4} txtT[j_chunk, b_blk].T @ W1[i, j_chunk, :, :]
      acc[b_blk] = psum * img[b_blk, i] + acc[b_blk]   # one fused DVE op
W1 streams through as the moving operand in f16 (1 cycle/row at N=512 =
full PE rate, half the HBM bytes of f32); txtT is the stationary operand
(f16 -> fast weight load). W1 is pre-scaled by 512 on the host (sigma ~1,
no f16 subnormals); the 1/512 is folded into the per-partition img scale.

DMA layout: every SBUF row is >= 2 KB so the DMA moves full-size packets
(the f32 baseline was packet-rate-bound at ~1 KB/packet):
 - W1 per i as one [128, 2048] f16 tile (row r = W1[i, {r,128+r,256+r,384+r}, :])
 - txt as one [128, 2048] f16 tile (row r = txtT[{r,128+r,256+r,384+r}, :])
 - img as one [128, 256] f32 tile (row r = img[{r,128+r,256+r,384+r}, :]/512)

Epilogue is a single fused InstTensorScalarPtr per (i, b_blk) on the Vector
engine: acc = (psum * img_scalar) + acc, reading PSUM directly.

Startup: dummy matmuls on a zeroed tile keep the PE HAM un-throttled
(2.4 GHz) while the first input DMAs land; txt is first in the DMA queues.
"""

import numpy as np

import concourse.bass as bass
import concourse.tile as tile
from concourse import bacc, mybir
from concourse.bass_utils import run_bass_kernel_spmd

B, D, H = 512, 512, 512
N_CORES = 8
I_PER_CORE = D // N_CORES          # 64
N_BBLK = B // 128                  # 4
N_JCHUNK = D // 128                # 4
EPS = 1e-12
W1_SCALE = 512.0                   # power of two: exact in fp, folded into img

F32 = mybir.dt.float32
F16 = mybir.dt.float16

N_WARM = 12                        # dummy matmuls to keep HAM warm at start

_CACHE = {}


def _l2norm(x: np.ndarray) -> np.ndarray:
    n = np.sqrt(np.sum(x * x, axis=1, keepdims=True, dtype=np.float32))
    return (x / np.maximum(n, np.float32(EPS))).astype(np.float32)


def build_nc():
    """Build the per-core Bass program (SPMD: same program, per-core data)."""
    nc = bacc.Bacc(
        "TRN2",
        target_bir_lowering=False,
        debug=False,
        num_devices=N_CORES,
    )

    txt_p = nc.dram_tensor("txt_p", [128, N_JCHUNK * B], F16, kind="ExternalInput").ap()
    img_p = nc.dram_tensor(
        "img_p", [128, N_BBLK * I_PER_CORE], F32, kind="ExternalInput"
    ).ap()
    w1_s = nc.dram_tensor(
        "w1_s", [I_PER_CORE, 128, N_JCHUNK * H], F16, kind="ExternalInput"
    ).ap()
    yp = nc.dram_tensor("yp", [B, H], F16, kind="ExternalOutput").ap()

    with tile.TileContext(nc) as tc:
        with (
            tc.tile_pool(name="warm", bufs=1) as warmp,
            tc.tile_pool(name="warmps", bufs=2, space=bass.MemorySpace.PSUM) as warmpsp,
            tc.tile_pool(name="const", bufs=1) as constp,
            tc.tile_pool(name="w1", bufs=8) as w1p,
            tc.tile_pool(name="accs", bufs=1) as accp,
            tc.tile_pool(name="ps", bufs=6, space=bass.MemorySpace.PSUM) as psump,
        ):
            # txt stationary first in the DMA queues: the first matmuls need
            # it before any W1. Halves (2 KB rows) so the first matmul group
            # only waits for chunks c0/c1 + the first W1 half.
            txt_sb = constp.tile([128, N_JCHUNK * B], F16, tag="txt", name="txt_sb")
            nc.sync.dma_start(txt_sb[:, : 2 * B], txt_p[:, : 2 * B])

            # First i-slab of W1 interleaved (halves hold chunks c0/c1 and
            # c2/c3 respectively).
            w1t0 = w1p.tile([128, N_JCHUNK * H], F16, tag="w1", name="w1p0")
            nc.sync.dma_start(w1t0[:, : 2 * H], w1_s[0, :, : 2 * H])
            nc.sync.dma_start(txt_sb[:, 2 * B :], txt_p[:, 2 * B :])
            nc.sync.dma_start(w1t0[:, 2 * H :], w1_s[0, :, 2 * H :])

            img_sb = constp.tile(
                [128, N_BBLK * I_PER_CORE], F32, tag="img", name="img_sb"
            )
            nc.sync.dma_start(img_sb[:], img_p[:, :])

            acc = [
                accp.tile([128, H], F32, tag=f"acc{bb}", name=f"acc{bb}")
                for bb in range(N_BBLK)
            ]
            # f16 staging for the final partial: halves the output DMA and
            # keeps the f32 accumulator out of the tail critical path.
            yp_sb = [
                accp.tile([128, H], F16, tag=f"yp{bb}", name=f"yp{bb}")
                for bb in range(N_BBLK)
            ]

            # Warm-up: dummy matmuls on a zeroed tile keep the PE busy (and
            # the HAM un-throttled) while the real input DMAs land. They sit
            # ahead of the real matmuls on the tensor queue and have no data
            # dependencies beyond the one memset.
            wz = warmp.tile([128, 384], F16, tag="wz", name="warm_zero")
            nc.vector.memset(wz[:], 0.0)
            for k in range(N_WARM):
                wps = warmpsp.tile([128, 256], F32, tag="wps")
                nc.tensor.matmul(
                    wps[:], wz[:, :128], wz[:, 128:384], start=True, stop=True
                )

            for i in range(I_PER_CORE):
                if i == 0:
                    w1t = w1t0
                else:
                    w1t = w1p.tile([128, N_JCHUNK * H], F16, tag="w1", name="w1")
                    nc.sync.dma_start(w1t[:], w1_s[i])
                for bb in range(N_BBLK):
                    ps = psump.tile([128, H], F32, tag="ps")
                    for c in range(N_JCHUNK):
                        nc.tensor.matmul(
                            ps[:],
                            txt_sb[:, c * B + bb * 128 : c * B + (bb + 1) * 128],
                            w1t[:, c * H : (c + 1) * H],
                            start=(c == 0),
                            stop=(c == N_JCHUNK - 1),
                        )
                    sc = img_sb[:, bb * I_PER_CORE + i : bb * I_PER_CORE + i + 1]
                    if i == 0:
                        # acc = psum * img  (fused, Vector engine)
                        nc.vector.tensor_scalar_mul(acc[bb][:], ps[:], sc)
                    elif i == I_PER_CORE - 1:
                        # Final partial goes straight to the f16 staging tile.
                        nc.vector.scalar_tensor_tensor(
                            yp_sb[bb][:], ps[:], sc, acc[bb][:],
                            mybir.AluOpType.mult, mybir.AluOpType.add,
                        )
                        nc.sync.dma_start(
                            yp[bb * 128 : (bb + 1) * 128, :], yp_sb[bb][:]
                        )
                    else:
                        # acc = (psum * img) + acc  (one fused DVE op)
                        nc.vector.scalar_tensor_tensor(
                            acc[bb][:], ps[:], sc, acc[bb][:],
                            mybir.AluOpType.mult, mybir.AluOpType.add,
                        )

    nc.compile()
    return nc


def make_in_maps(image_embeds, text_embeds, W1):
    imgn = _l2norm(np.asarray(image_embeds, np.float32)) * np.float32(1.0 / W1_SCALE)
    txtn = _l2norm(np.asarray(text_embeds, np.float32))
    # txt packed: row r = [txtT[r], txtT[128+r], txtT[256+r], txtT[384+r]]
    txt_t = np.ascontiguousarray(txtn.T).astype(np.float16)          # [D, B]
    txt_p = np.ascontiguousarray(
        txt_t.reshape(N_JCHUNK, 128, B).transpose(1, 0, 2).reshape(128, N_JCHUNK * B)
    )
    W1r = (np.asarray(W1, np.float32).reshape(D, D, H) * np.float32(W1_SCALE)).astype(
        np.float16
    )
    in_maps = []
    for c in range(N_CORES):
        # img packed per core: row r = [img[r, Ic], img[128+r, Ic], ...]
        ic = np.ascontiguousarray(imgn[:, c * I_PER_CORE : (c + 1) * I_PER_CORE])
        img_pk = np.ascontiguousarray(
            ic.reshape(N_BBLK, 128, I_PER_CORE)
            .transpose(1, 0, 2)
            .reshape(128, N_BBLK * I_PER_CORE)
        )
        # W1 per i: row r = [W1[i, r, :], W1[i, 128+r, :], W1[i, 256+r, :], W1[i, 384+r, :]]
        w1c = W1r[c * I_PER_CORE : (c + 1) * I_PER_CORE]             # [64, D, H]
        w1pk = np.ascontiguousarray(
            w1c.reshape(I_PER_CORE, N_JCHUNK, 128, H)
            .transpose(0, 2, 1, 3)
            .reshape(I_PER_CORE, 128, N_JCHUNK * H)
        )
        in_maps.append({"txt_p": txt_p, "img_p": img_pk, "w1_s": w1pk})
    return in_maps


def run_device(in_maps, trace=False, **kw):
    if "nc" not in _CACHE:
        _CACHE["nc"] = build_nc()
    return run_bass_kernel_spmd(
        _CACHE["nc"], in_maps, list(range(N_CORES)), trace=trace, **kw
    )


def finish_host(results, b1, W2, b2):
    Y = np.zeros((B, H), np.float32)
    for c in range(N_CORES):
        Y += results[c]["yp"].astype(np.float32)
    h = np.maximum(Y + np.asarray(b1, np.float32), np.float32(0.0))
    out = h @ np.asarray(W2, np.float32) + np.asarray(b2, np.float32)
    return out.astype(np.float32)


def kernel(image_embeds, text_embeds, W1, b1, W2, b2):
    in_maps = make_in_maps(image_embeds, text_embeds, W1)
    res = run_device(in_maps, trace=False)
    return finish_host(res.results, b1, W2, b2)
